# revision 36
# baseline (speedup 1.0000x reference)
"""RNN-T joint network (Conformer transducer) kernel for Trainium2.

Computes out[b,t,u,v] = (enc[b,t,:] @ W[:, :D].T)[v] + (dec[b,u,:] @ W[:, D:].T)[v]
i.e. the broadcast-sum decomposition of cat(enc, dec) @ W.T without
materialising the (B,T,U,2D) concat.

Sharding: the (B*T) = 1024 grid rows are split across 8 NeuronCores
(cores 0-3 take b=0, cores 4-7 take b=1, 128 t-rows each). W is
replicated. Each core emits its own 32 MB fp16 slab; the host
reassembles the full fp32 (B,T,U,V) tensor. fp16 output keeps the
max-relative error ~3e-3, well inside the 2e-2 budget, and halves the
HBM write traffic.

Per-core structure (all matmul operands bf16 - the PE runs bf16 at
1 cycle/column vs 2 for fp16):
  1. enc_proj / dec_proj bf16 matmuls on the TensorEngine (K=512 in 4
     chunks). Each K-chunk's lhsT and rhs live in one packed SBUF tile
     fed by a single DMA.
  2. Projections are rounded to bf16 (enc_hi / dec2; dec2 duplicated so
     one FD=1024 DVE add covers a pair of tiles). A dif4 matmul also
     produces enc_dif[t] = enc_hi[t] - enc_hi[t-4].
  3. Tiles are processed in pairs. Per t, a one-hot matmul broadcasts
     row t across all 128 PSUM partitions; the "selector" is column t
     of the identity matrix expanded by a stride-0 access pattern, so
     no selector tensor is ever loaded. Every main-loop matmul keeps
     the same (128,128) PE tile config: a config switch stalls the
     array, poisoning the HAM activity window and de-rating the PE
     clock from 2.4 to 1.2 GHz.
  4. v-lower halves go to a rotating PSUM tile; v-upper halves live in
     persistent PSUM "chains" that hold dec + enc(t) and accumulate
     enc_dif per step (re-seeded with dec via identity matmuls every 4
     groups to cap the bf16 rounding walk). This removes the per-tile
     dec matmul, keeping the PE off the critical path.
  5. The drain is split across two engines reading disjoint PSUM tiles
     (the Tile framework chains accessors of a shared tile, which
     would serialise them): VectorE computes fp16(ps_lo + dec2),
     ScalarE copies fp16(chain), FD=1024 per instruction, into
     separate SBUF tiles (again to avoid accessor chaining).
  6. Four t-tiles form one DMA group written by two 512 KB HWDGE DMAs
     (4 KB per-partition descriptors); the host un-permutes the group
     layout while upcasting to fp32.

The kernel is wire-limited: ~32 MB of fp16 output per core at the
~358 GB/s per-core HBM cap, plus ~2.4 MB of inputs.
"""

import numpy as np

import bass_rust
import concourse.bass as bass
import concourse.tile as tile
from concourse import bacc
from concourse import mybir
from concourse.bass_utils import run_bass_kernel_spmd

_vec_pair = bass_rust.VecI64Pair

B, T, U, D, V = 2, 512, 128, 512, 1024
N_CORES = 8
T_LOC = (B * T) // N_CORES  # 128 t-rows per core
PKW = 128 + V  # packed chunk width: [lhsT column block | rhs row block]
TG = 4  # t-tiles per output DMA group

F32 = mybir.dt.float32
F16 = mybir.dt.float16
BF16 = mybir.dt.bfloat16


def _build_program() -> bass.Bass:
    nc = bacc.Bacc("TRN2", debug=False, num_devices=N_CORES)

    # PACK[kc] = [encT chunk kc | WT chunk kc]        for kc in 0..3
    #          = [decT chunk kc-4 | WT chunk kc]      for kc in 4..7
    PACK = nc.dram_tensor("PACK", [8, 128, PKW], BF16, kind="ExternalInput").ap()
    IDNR = nc.dram_tensor("IDNR", [128, 128], BF16, kind="ExternalInput").ap()
    # DIFR[k, t] = dif4 matrix: enc_dif[t] = enc_hi[t] - enc_hi[t-4] (t >= 4)
    DIFR = nc.dram_tensor("DIFR", [128, 128], BF16, kind="ExternalInput").ap()
    # out[grp, half, u, tt, vl] = true_out[TG*grp + tt, u, 512*half + vl];
    # the v-halves are separate so the DVE and ACT drain into separate SBUF
    # tiles (two writers to one tile would serialise); host un-permutes.
    OUT = nc.dram_tensor(
        "out", [T_LOC // TG, 2, U, TG, 512], F16, kind="ExternalOutput"
    ).ap()

    with tile.TileContext(nc) as tc:
        with (
            tc.tile_pool(name="const", bufs=1) as cpool,
            tc.tile_pool(name="pmain", bufs=2, space="PSUM") as pmain,
            tc.tile_pool(name="pchain", bufs=1, space="PSUM") as pchain,
            tc.tile_pool(name="outp", bufs=12) as opool,
        ):
            # ---- inputs to SBUF ----
            # dec chunks (4-7) first: the dec projection runs first on the PE.
            pk = [None] * 8
            for kc in (4, 5, 6, 7, 0, 1, 2, 3):
                tl = cpool.tile([128, PKW], BF16, tag=f"pk{kc}")
                nc.sync.dma_start(out=tl[:], in_=PACK[kc])
                pk[kc] = tl
            idn = cpool.tile([128, 128], BF16, tag="idn")
            nc.sync.dma_start(out=idn[:], in_=IDNR)
            dif = cpool.tile([128, 128], BF16, tag="dif")
            nc.sync.dma_start(out=dif[:], in_=DIFR)

            # ---- dec_proj = decT.T @ W_decT : (U, V) ----
            dec_ps = pmain.tile([128, 2, 512], F32, tag="ps")
            for vh in range(2):
                for kc in range(4):
                    nc.tensor.matmul(
                        dec_ps[:, vh, :],
                        lhsT=pk[4 + kc][:, 0:128],
                        rhs=pk[4 + kc][:, 128 + 512 * vh : 128 + 512 * (vh + 1)],
                        start=(kc == 0),
                        stop=(kc == 3),
                    )
            # bf16 copy on the ScalarEngine: warms the ACT table and keeps
            # the DVE free; duplicate for pair-wide DVE adds.
            dec2 = cpool.tile([128, 2, V], BF16, tag="dec2")
            nc.scalar.copy(out=dec2[:, 0, :], in_=dec_ps[:])
            nc.vector.tensor_copy(out=dec2[:, 1, :], in_=dec2[:, 0, :])

            # ---- enc_proj = encT.T @ W_encT : (T_LOC, V) ----
            enc_ps = pmain.tile([128, 2, 512], F32, tag="ps")
            for vh in range(2):
                for kc in range(4):
                    nc.tensor.matmul(
                        enc_ps[:, vh, :],
                        lhsT=pk[kc][:, 0:128],
                        rhs=pk[kc][:, 128 + 512 * vh : 128 + 512 * (vh + 1)],
                        start=(kc == 0),
                        stop=(kc == 3),
                    )
            enc_hi = cpool.tile([128, V], BF16, tag="ehi")
            nc.vector.tensor_copy(out=enc_hi[:], in_=enc_ps[:])

            # enc_dif[t] = enc_hi[t] - enc_hi[t-4]: lets the v-upper PSUM
            # chains accumulate a diff per step instead of re-adding dec.
            dif_ps = pmain.tile([128, 2, 512], F32, tag="ps")
            for vh in range(2):
                nc.tensor.matmul(
                    dif_ps[:, vh, :],
                    lhsT=dif[:],
                    rhs=enc_hi[:, 512 * vh : 512 * (vh + 1)],
                    start=True,
                    stop=True,
                )
            enc_dif = cpool.tile([128, V], BF16, tag="edif")
            nc.vector.tensor_copy(out=enc_dif[:], in_=dif_ps[:])

            # Persistent v-upper PSUM chains: chain[p][:, tt, :] holds
            # dec + enc(t) for t = 4*i + 2*p + tt; each step accumulates
            # enc_dif via a one-hot matmul instead of re-adding dec.
            chain0 = pchain.tile([128, 2, 512], F32, tag="chain0")
            chain1 = pchain.tile([128, 2, 512], F32, tag="chain1")
            chains = [chain0, chain1]

            # ---- main loop: TG t-tiles per DMA group, pairs per PSUM tile ----
            for grp in range(T_LOC // TG):
                ob_dve = opool.tile([128, TG, 512], F16, tag="obd")
                ob_act = opool.tile([128, TG, 512], F16, tag="oba")
                # Re-seed the chains every 4 groups to cap the accumulated
                # bf16 rounding walk of the dif steps.
                init = grp % 4 == 0
                for half in range(TG // 2):
                    t0 = TG * grp + 2 * half
                    # v-lower halves live in a rotating PSUM tile drained by
                    # the DVE only; v-upper halves live in the persistent
                    # chain drained by the ACT only. (Separate tiles per
                    # engine: the Tile framework chains accessors of a tile,
                    # so sharing one would serialise the two drain engines.)
                    ps_lo = pmain.tile([128, 2, 512], F32, tag="ps")
                    ps_hi = chains[half % 2]
                    for tt in range(2):
                        # 128-row one-hot selector = column t of the identity,
                        # broadcast across 128 weight columns via a stride-0
                        # AP (no 4 MB selector tensor needed). Every main-loop
                        # matmul keeps the same (128,128) tile config, so the
                        # PE array never pays a tile-reconfig stall (which
                        # would poison the HAM activity window and de-rate the
                        # clock to 1.2 GHz).
                        t = t0 + tt
                        sel_ap = idn[:, t : t + 1].copy()
                        part = sel_ap.ap.to_list()[0]
                        sel_ap.ap = _vec_pair([part, [0, 128]])
                        # v-lower half: enc broadcast only (dec added by DVE).
                        nc.tensor.matmul(
                            ps_lo[:, tt, :],
                            lhsT=sel_ap,
                            rhs=enc_hi[:, 0:512],
                            start=True,
                            stop=True,
                            tile_position=(0, 0),
                            skip_group_check=True,
                        )
                        # v-upper half: first visit initialises the chain with
                        # enc(t); later visits accumulate enc(t) - enc(t-4).
                        nc.tensor.matmul(
                            ps_hi[:, tt, :],
                            lhsT=sel_ap,
                            rhs=(enc_hi if init else enc_dif)[:, 512:1024],
                            start=init,
                            stop=not init,
                            tile_position=(0, 0),
                            skip_group_check=True,
                        )
                    if init:
                        # Chain init: accumulate dec once via identity matmuls
                        # (back to back so the PE loads the idn weights once).
                        for tt in range(2):
                            nc.tensor.matmul(
                                ps_hi[:, tt, :],
                                lhsT=idn[:],
                                rhs=dec2[:, 0, 512:1024],
                                start=False,
                                stop=True,
                                tile_position=(0, 0),
                                skip_group_check=True,
                            )
                    # Drain both tiles of the pair in one instruction per
                    # engine: DVE adds dec to the lower halves, ACT copies
                    # the upper halves (dec already in PSUM).
                    nc.vector.tensor_add(
                        out=ob_dve[:, 2 * half : 2 * half + 2, :],
                        in0=ps_lo[:],
                        in1=dec2[:, :, 0:512],
                    )
                    nc.scalar.copy(
                        out=ob_act[:, 2 * half : 2 * half + 2, :],
                        in_=ps_hi[:],
                    )
                nc.sync.dma_start(out=OUT[grp, 0], in_=ob_dve[:])
                nc.sync.dma_start(out=OUT[grp, 1], in_=ob_act[:])
    nc.compile()
    return nc


def _to_bf16(a: np.ndarray) -> np.ndarray:
    # numpy has no bfloat16; round-to-nearest-even to bf16 kept in a uint16
    # view, which is what run_bass_kernel_spmd expects for BF16 tensors.
    try:
        import ml_dtypes

        return a.astype(ml_dtypes.bfloat16)
    except ImportError:
        x = a.astype(np.float32).view(np.uint32)
        rounded = (x + 0x7FFF + ((x >> 16) & 1)) >> 16
        return rounded.astype(np.uint16)


_PROGRAM = None


def _get_program() -> bass.Bass:
    global _PROGRAM
    if _PROGRAM is None:
        _PROGRAM = _build_program()
    return _PROGRAM


def _make_in_maps(inputs):
    enc = np.asarray(inputs["encoder_outputs"], dtype=np.float32)
    dec = np.asarray(inputs["decoder_outputs"], dtype=np.float32)
    W = np.asarray(inputs["W"], dtype=np.float32)
    WT = np.ascontiguousarray(W.T)  # (2D, V)
    IDN = _to_bf16(np.eye(128, dtype=np.float32))
    # DIF[k, t] = 1 if k == t, -1 if k == t - 4: one matmul turns enc_hi
    # into the per-step chain increments enc_hi[t] - enc_hi[t-4].
    dif = np.eye(128, dtype=np.float32)
    for t in range(4, 128):
        dif[t - 4, t] = -1.0
    DIF = _to_bf16(dif)
    in_maps = []
    for c in range(N_CORES):
        b = c // (N_CORES // B)
        t0 = (c % (N_CORES // B)) * T_LOC
        encT = enc[b, t0 : t0 + T_LOC, :].T  # (D, T_LOC)
        decT = dec[b].T  # (D, U)
        pack = np.empty((8, 128, PKW), np.float32)
        for kc in range(4):
            pack[kc, :, :128] = encT[128 * kc : 128 * (kc + 1), :]
            pack[kc, :, 128:] = WT[128 * kc : 128 * (kc + 1), :]
        for kc in range(4, 8):
            pack[kc, :, :128] = decT[128 * (kc - 4) : 128 * (kc - 3), :]
            pack[kc, :, 128:] = WT[128 * kc : 128 * (kc + 1), :]
        in_maps.append({"PACK": _to_bf16(pack), "IDNR": IDN, "DIFR": DIF})
    return in_maps


def _unpermute(slab: np.ndarray) -> np.ndarray:
    # (T_LOC//TG, 2, U, TG, 512) -> (T_LOC, U, V)
    return slab.transpose(0, 3, 2, 1, 4).reshape(T_LOC, U, V)


def _assemble(results) -> np.ndarray:
    out = np.empty((B, T, U, V), np.float32)
    for c in range(N_CORES):
        b = c // (N_CORES // B)
        t0 = (c % (N_CORES // B)) * T_LOC
        out[b, t0 : t0 + T_LOC] = _unpermute(np.asarray(results[c]["out"]))
    return out


def _run(inputs, **spmd_kwargs):
    nc = _get_program()
    in_maps = _make_in_maps(inputs)
    res = run_bass_kernel_spmd(nc, in_maps, core_ids=list(range(N_CORES)), **spmd_kwargs)
    return _assemble(res.results), res


def kernel(**inputs) -> np.ndarray:
    out, _ = _run(inputs)
    return out
